# revision 17
# baseline (speedup 1.0000x reference)
"""Relative-position (Transformer-XL style) attention on 8 trn2 NeuronCores.

Sharding: data-parallel over batch (2) x tensor-parallel over heads (16/4=4
heads per core). Each core computes its 4 heads' attention for its batch and
a partial output projection (Wo rows for its heads); the host sums the 4
partials per batch (the "all-reduce").

Layout notes (per core, T=2048, D=1024, HC=4 local heads, HD=64):
  - q/k/pe are computed TRANSPOSED: [256, T] with head-dim on partitions,
    stored as 2 tiles [128, T] (head pair hp: heads 2hp at partitions 0-63,
    2hp+1 at 64-127).  Biases are per-partition there and folded into the
    PSUM->SBUF copies.  r_w/r_r biases are combined with bq on the host.
  - relative_shift: scores R = (q+r_r)@peT are written to a padded DRAM
    buffer P[T, T+1] (col 0 zeroed); the shifted matrix is exactly
    P_flat[(i+1)*T + j], i.e. a re-strided read (row stride T over a buffer
    of row pitch T+1).
  - content scores go straight to PSUM; the shifted rel scores are
    accumulated on top with an identity matmul (PE), then one fused
    exp(0.125*x) pass on ScalarE produces P (prob) tiles in bf16.
  - P@V: P tiles are [keys, queries] so v-as-lhsT matmuls produce
    outT [64, q] directly; a ones column appended to v also produces the
    softmax row-sums.  outT is normalized by 1/rowsum (partition-broadcast).
  - final: out_part[t, :] = sum_dc outT[dc, t] * Wo[dc, :] (+ bo on the
    g==0 cores), DMA'd out; host sums partials per batch.

All matmul operands are bf16 (PE streams at full rate, lower energy than
float32r); accumulation stays fp32 in PSUM.  fast=False is a full-fp32
reference build.
"""

import math
from contextlib import ExitStack

import ml_dtypes
import numpy as np

import concourse.bass as bass
import concourse.mybir as mybir
import concourse.tile as tile
from concourse.tile_rust import add_dep_helper
from concourse import bacc
from concourse.bass_utils import run_bass_kernel_spmd
from concourse.masks import make_identity

F32 = mybir.dt.float32
BF16 = mybir.dt.bfloat16
FP8 = mybir.dt.float8e3          # e3m4: 4 mantissa bits, range +-30
AF = mybir.ActivationFunctionType

B, T, D, NH = 2, 2048, 1024, 16
HD = D // NH              # 64 head dim
HC = 4                    # heads per core
DC = HC * HD              # 256 cols per core
NCORES = 8
KC = D // 128             # 8 contraction chunks for projections
NB = T // 512             # 4 key blocks of 512
QT = T // 128             # 16 q tiles
MAX_TS = 10000.0


def build_program(fast=True):
    P_DT = BF16 if fast else F32   # prob / v / pT dtype
    R_DT = BF16                    # rel-score DRAM roundtrip dtype

    nc = bacc.Bacc("TRN2", target_bir_lowering=False, debug=False,
                   num_devices=NCORES)

    IN_DT = BF16 if fast else F32  # streamed projection operand dtype
    # score-matmul operands (q/k/pe).  bf16: fp8 was measured to push the
    # result past the 2e-2 error gate (scores see ~4% noise amplified
    # through exp), and did not reduce the power throttle either.
    A_DT = BF16 if fast else F32   # engine-produced activation dtype
    xT = nc.dram_tensor("xT", [D, T], IN_DT, kind="ExternalInput")
    yT = nc.dram_tensor("yT", [D, T], IN_DT, kind="ExternalInput")
    sinT = nc.dram_tensor("sinT", [D, T], IN_DT, kind="ExternalInput")
    wq = nc.dram_tensor("wq", [D, DC], IN_DT, kind="ExternalInput")
    wk = nc.dram_tensor("wk", [D, DC], IN_DT, kind="ExternalInput")
    wv = nc.dram_tensor("wv", [D, DC], IN_DT, kind="ExternalInput")
    wp = nc.dram_tensor("wp", [D, DC], IN_DT, kind="ExternalInput")
    wo = nc.dram_tensor("wo", [DC, D], F32, kind="ExternalInput")
    bqw = nc.dram_tensor("bqw", [DC, 1], F32, kind="ExternalInput")
    bqr = nc.dram_tensor("bqr", [DC, 1], F32, kind="ExternalInput")
    bkb = nc.dram_tensor("bkb", [DC, 1], F32, kind="ExternalInput")
    bpb = nc.dram_tensor("bpb", [DC, 1], F32, kind="ExternalInput")
    bvb = nc.dram_tensor("bvb", [DC, 1], F32, kind="ExternalInput")
    bob = nc.dram_tensor("bob", [128, D], F32, kind="ExternalInput")
    out = nc.dram_tensor("out", [T, D], F32, kind="ExternalOutput")

    with tile.TileContext(nc) as tc, ExitStack() as ctx:
        persist = ctx.enter_context(tc.tile_pool(name="persist", bufs=1))
        dpool = ctx.enter_context(tc.tile_pool(name="dram", bufs=1,
                                               space="DRAM"))

        # ---- constants / biases -------------------------------------------
        ident_p = persist.tile([128, 128], P_DT, tag="ident_p")
        make_identity(nc, ident_p[:])
        ident_f = persist.tile([128, 128], F32, tag="ident_f")
        make_identity(nc, ident_f[:])

        def load_bias_pair(name, dram):
            ts_ = [persist.tile([128, 1], F32, name=f"{name}{i}",
                                tag=f"{name}{i}") for i in range(2)]
            for i in range(2):
                nc.scalar.dma_start(ts_[i][:], dram[i * 128:(i + 1) * 128, 0:1])
            return ts_

        bqw_sb = load_bias_pair("bqw", bqw)
        bqr_sb = load_bias_pair("bqr", bqr)
        bk_sb = load_bias_pair("bk", bkb)
        bp_sb = load_bias_pair("bp", bpb)
        bv_sb = load_bias_pair("bv", bvb)
        bo_sb = persist.tile([128, D], F32, tag="bo")
        nc.scalar.dma_start(bo_sb[:], bob[:, :])

        wo_sb = [persist.tile([HD, D], P_DT if fast else F32,
                              name=f"wo{i}", tag=f"wo{i}")
                 for i in range(HC)]
        with tc.tile_pool(name="wotmp", bufs=2) as wotmp:
            for i in range(HC):
                wo_f = wotmp.tile([HD, D], F32, name=f"wof{i}", tag="wof")
                nc.scalar.dma_start(wo_f[:], wo[i * HD:(i + 1) * HD, :])
                nc.scalar.copy(wo_sb[i][:], wo_f[:])

        # ---- DRAM scratch for the relative-shift roundtrip ----------------
        pbufs = [dpool.tile([T, T + 1], R_DT, name=f"pbuf{h}",
                            tag=f"pbuf{h}") for h in range(HC)]
        pbuf_h = [pb[:].tensor for pb in pbufs]

        w_insts = {}             # (h, qt) -> rel-score write DMA

        # ---- persistent activations ---------------------------------------
        cqT = [persist.tile([128, T], A_DT, name=f"cqT{i}", tag=f"cqT{i}")
               for i in range(2)]
        rqT = [persist.tile([128, T], A_DT, name=f"rqT{i}", tag=f"rqT{i}")
               for i in range(2)]
        kT = [persist.tile([128, T], A_DT, name=f"kT{i}", tag=f"kT{i}")
              for i in range(2)]
        peT = [persist.tile([128, T], A_DT, name=f"peT{i}", tag=f"peT{i}")
               for i in range(2)]
        # v with a ones column appended per head: [keys, (kc, head, 65)];
        # the ones row makes P@V also produce the softmax row-sums.
        VW = HC * (HD + 1)
        v_sb = persist.tile([128, QT * VW], P_DT, tag="v")
        O_DT = P_DT if fast else F32
        outT = [persist.tile([HD, T], O_DT, name=f"outT{i}", tag=f"outT{i}")
                for i in range(HC)]

        # ---- rel-score (R) infrastructure ---------------------------------
        # created before projections so R units can interleave into them
        rsb_pool = ctx.enter_context(tc.tile_pool(name="rsb", bufs=2))
        ps_r = ctx.enter_context(tc.tile_pool(name="psr", bufs=1, space="PSUM"))
        o_pool = ctx.enter_context(tc.tile_pool(name="osb", bufs=2))

        def r_unit(hp2, qt, hl, in_loop=False):
            """rel scores for one (head, q-tile) -> padded DRAM buf.

            in_loop: emitted while the (ACT-saturated) score loop runs, so
            both PSUM->SBUF copies go to VectorE instead of one on ScalarE.
            """
            h = 2 * hp2 + hl
            pb, pe_ = 64 * hl, 64 * (hl + 1)
            # [128, T+1]: col 0 is the zero pad column of the shift buffer,
            # so the write covers whole pitch-(T+1) rows contiguously.
            rsb_t = rsb_pool.tile([128, T + 1], R_DT, name="rsb_t", tag="rsb")
            nc.vector.memset(rsb_t[:, 0:1], 0.0)
            for jb in range(2):
                rp = ps_r.tile([128, 1024], F32, name="rp", tag="rp")
                for i in range(2):
                    nc.tensor.matmul(
                        rp[:, i * 512:(i + 1) * 512],
                        rqT[hp2][pb:pe_, qt * 128:(qt + 1) * 128],
                        peT[hp2][pb:pe_,
                                 (2 * jb + i) * 512:(2 * jb + i + 1) * 512],
                        start=True, stop=True)
                if jb == 0 and not in_loop:
                    nc.scalar.copy(
                        rsb_t[:, 1 + jb * 1024:1 + (jb + 1) * 1024], rp[:])
                else:
                    nc.vector.tensor_copy(
                        rsb_t[:, 1 + jb * 1024:1 + (jb + 1) * 1024], rp[:])
            dst = bass.AP(tensor=pbuf_h[h], offset=qt * 128 * (T + 1),
                          ap=[[T + 1, 128], [1, T + 1]])
            w_insts[(h, qt)] = nc.gpsimd.dma_start(dst, rsb_t[:])

        r_done = [0, 0]          # next unemitted qt per head pair

        def ensure_r(hp2, upto_qt, in_loop=False):
            while r_done[hp2] <= min(upto_qt, QT - 1):
                for hl in range(2):
                    r_unit(hp2, r_done[hp2], hl, in_loop=in_loop)
                r_done[hp2] += 1

        def r_stepper(hp2):
            def step():
                if r_done[hp2] < QT:
                    for hl in range(2):
                        r_unit(hp2, r_done[hp2], hl)
                    r_done[hp2] += 1
            return step

        # ---- phase B: projections -----------------------------------------
        def proj_T(src_dram, w_dram, dests, interleave=None):
            """dest[mc][:, :] = (W[:, mc]^T @ src) + bias  (transposed proj).

            Stream kept resident; two nb-passes over 2x[128,1024] PSUM tiles
            (6 banks with bufs=3, leaving room for concurrent R units).
            """
            with ExitStack() as c2:
                wpool = c2.enter_context(tc.tile_pool(name="wproj", bufs=1))
                spool = c2.enter_context(tc.tile_pool(name="sproj", bufs=1))
                pspool = c2.enter_context(
                    tc.tile_pool(name="psproj", bufs=3, space="PSUM"))
                w_t = [wpool.tile([128, DC], IN_DT, name=f"w{k}", tag=f"w{k}")
                       for k in range(KC)]
                s_t = [spool.tile([128, T], IN_DT, name=f"s{k}", tag=f"s{k}")
                       for k in range(KC)]
                for k in range(KC):
                    nc.scalar.dma_start(w_t[k][:],
                                        w_dram[k * 128:(k + 1) * 128, :])
                    eng = nc.sync if k % 2 == 0 else nc.scalar
                    eng.dma_start(s_t[k][:], src_dram[k * 128:(k + 1) * 128, :])
                for p2 in range(2):
                    ps = [pspool.tile([128, 1024], F32, name=f"ps{mc}",
                                      tag="ps") for mc in range(2)]
                    for k in range(KC):
                        for mc in range(2):
                            for i in range(2):
                                nb = 2 * p2 + i
                                nc.tensor.matmul(
                                    ps[mc][:, i * 512:(i + 1) * 512],
                                    w_t[k][:, mc * 128:(mc + 1) * 128],
                                    s_t[k][:, nb * 512:(nb + 1) * 512],
                                    start=(k == 0), stop=(k == KC - 1))
                        if interleave is not None:
                            interleave()
                    for mc in range(2):
                        for d_tile, d_bias in dests:
                            nc.scalar.activation(
                                d_tile[mc][:, 2 * p2 * 512:(2 * p2 + 2) * 512],
                                ps[mc][:], AF.Identity,
                                bias=d_bias[mc][:], scale=1.0)

        proj_T(xT, wq, [(cqT, bqw_sb), (rqT, bqr_sb)])
        proj_T(sinT, wp, [(peT, bp_sb)])
        proj_T(yT, wk, [(kT, bk_sb)], interleave=r_stepper(0))

        # vT = (Wv^T @ yT) + bv, then PE-transpose into natural [t, dc] layout
        with ExitStack() as c2:
            vtpool = c2.enter_context(tc.tile_pool(name="vtp", bufs=1))
            vT = [vtpool.tile([128, T], F32, name=f"vT{i}", tag=f"vT{i}")
                  for i in range(2)]
            proj_T(yT, wv, [(vT, bv_sb)], interleave=r_stepper(0))
            pst_v = c2.enter_context(
                tc.tile_pool(name="pstv", bufs=4, space="PSUM"))
            v_view = v_sb[:].rearrange("p (tt h d) -> p tt h d",
                                       h=HC, d=HD + 1)
            nc.vector.memset(v_view[:, :, :, HD:HD + 1], 1.0)
            for mc in range(2):
                for tt in range(QT):
                    tpv = pst_v.tile([128, 128], F32)
                    nc.tensor.matmul(tpv[:], vT[mc][:, tt * 128:(tt + 1) * 128],
                                     ident_f[:], is_transpose=True)
                    dst = v_view[:, tt, 2 * mc:2 * mc + 2, 0:HD]
                    srcv = tpv[:].rearrange("p (a b) -> p a b", b=HD)
                    if tt % 2 == 0:
                        nc.scalar.copy(dst, srcv)
                    else:
                        nc.vector.tensor_copy(dst, srcv)

        # ---- phase C: attention, software-pipelined -----------------------
        cctx = ExitStack()
        rst_pool = cctx.enter_context(tc.tile_pool(name="rst", bufs=40))
        pt_pool = cctx.enter_context(tc.tile_pool(name="pt", bufs=2))
        small_pool = cctx.enter_context(tc.tile_pool(name="small", bufs=2))
        ps_c = cctx.enter_context(tc.tile_pool(name="psc", bufs=2, space="PSUM"))
        ps_o = cctx.enter_context(tc.tile_pool(name="pso", bufs=2, space="PSUM"))

        def pv_mm(prev, po_t, kc):
            pt_p, plh, _ = prev
            nc.tensor.matmul(
                po_t[0:HD + 1, :],
                v_sb[:, kc * VW + plh * (HD + 1):kc * VW + (plh + 1) * (HD + 1)],
                pt_p[:, kc * 512:(kc + 1) * 512],
                start=(kc == 0), stop=(kc == 15))

        def normalize(prev, po_t):
            # NOTE: ACT Ln+Exp for 1/Z thrashes the ACT function-table (the
            # set loader alternates sets, ~1.3us per swap) — keep reciprocal
            # on DVE even though the [1,512] single-lane shape is slow.
            _, plh, pq0 = prev
            rec = small_pool.tile([1, 512], F32, name="rec", tag="rec")
            nc.vector.reciprocal(rec[:], po_t[HD:HD + 1, :])
            rec_b = small_pool.tile([HD, 512], F32, name="rec_b", tag="rec_b")
            nc.gpsimd.partition_broadcast(rec_b[:], rec[:])
            nc.vector.tensor_mul(outT[plh][:, pq0:pq0 + 512],
                                 po_t[0:HD, :], rec_b[:])

        def d_unit(qt):
            """partial output projection for one q-tile (uses a psc slot)."""
            wps = ps_c.tile([128, 1024], F32, name="wps", tag="cp")
            for nb2 in range(2):
                for lh2 in range(HC):
                    nc.tensor.matmul(
                        wps[:, nb2 * 512:(nb2 + 1) * 512],
                        outT[lh2][:, qt * 128:(qt + 1) * 128],
                        wo_sb[lh2][:, nb2 * 512:(nb2 + 1) * 512],
                        start=(lh2 == 0), stop=(lh2 == HC - 1))
            o_t = o_pool.tile([128, D], F32, name="o_t", tag="o_t")
            for nb2 in range(2):
                nc.vector.tensor_add(o_t[:, nb2 * 512:(nb2 + 1) * 512],
                                     wps[:, nb2 * 512:(nb2 + 1) * 512],
                                     bo_sb[:, nb2 * 512:(nb2 + 1) * 512])
            nc.gpsimd.dma_start(out[qt * 128:(qt + 1) * 128, :], o_t[:])

        # The P@V matmuls of the previous group interleave into the score
        # loop of the current one.  hl-outer order so consecutive qg groups
        # share one head's DRAM buf and each transposed read covers 1024 q
        # rows (2 groups).  The next head pair's R units and the output
        # projection of finished q ranges are spread into the loop.
        def issue_read(h2, half, kc):
            q0p = half * 1024
            rst = rst_pool.tile([128, 1024], R_DT, name=f"rst{kc}", tag="rst")
            src = bass.AP(tensor=pbuf_h[h2],
                          offset=(q0p + 1) * T + kc * 128,
                          ap=[[T, 1024], [1, 128]])
            rd = nc.sync.dma_start_transpose(rst[:], src)
            for qt2 in range(8 * half, min(8 * half + 9, QT)):
                add_dep_helper(rd.ins, w_insts[(h2, qt2)].ins,
                               reason="shifted read after rel write")
            return rst

        ensure_r(0, 8)
        groups = [(hp2, hl, qg)
                  for hp2 in range(2) for hl in range(2) for qg in range(NB)]
        prev = None
        rblk = [None] * 16
        nxt_rblk = None
        for gi, (hp2, hl, qg) in enumerate(groups):
            h = 2 * hp2 + hl
            lh = 2 * hp2 + hl            # local head index 0..3
            q0 = qg * 512
            pb, pe_ = 64 * hl, 64 * (hl + 1)
            if qg % 2 == 0:
                if nxt_rblk is not None:
                    rblk = nxt_rblk
                    nxt_rblk = None
                else:
                    ensure_r(hp2, 8 * (qg // 2) + 8)
                    rblk = [issue_read(h, qg // 2, kc) for kc in range(16)]
            # what the 2-group window after this one will need: at qg 0/1
            # prefetch (h, half 1); at qg 2/3 prefetch the next head's half 0.
            pf = None
            if qg in (0, 1):
                pf = (h, 1)
            elif gi + 4 - qg < len(groups):
                nhp, nhl, _ = groups[gi + 4 - qg]
                pf = (2 * nhp + nhl, 0)
            if pf is not None and nxt_rblk is None and qg in (0, 2):
                ensure_r(pf[0] // 2, 8 * pf[1] + 8, in_loop=True)
                nxt_rblk = []
            qoff = (qg % 2) * 512
            pt_t = pt_pool.tile([128, 16 * 512], P_DT, name="pt_t", tag="pt")
            po_t = None
            if prev is not None:
                po_t = ps_o.tile([128, 512], F32, name="po", tag="po")
            # during hp2=0 groups the DVE is busy with the next head pair's
            # rel-score copies, so the shifted-rel add runs as identity
            # matmuls on PE; during hp2=1 groups DVE is light, so the add
            # moves there (cuts PE stream cycles and PE power).
            dve_add = hp2 == 1
            for kb in range(8):
                cp = ps_c.tile([128, 1024], F32, name="cp", tag="cp")
                # content for both 512-halves first (weights: kT chunks),
                # then the rel accumulate, then the previous group's P@V.
                for i in range(2):
                    kc = 2 * kb + i
                    nc.tensor.matmul(
                        cp[:, i * 512:(i + 1) * 512],
                        kT[hp2][pb:pe_, kc * 128:(kc + 1) * 128],
                        cqT[hp2][pb:pe_, q0:q0 + 512],
                        start=True, stop=dve_add)
                if dve_add:
                    for i in range(2):
                        kc = 2 * kb + i
                        nc.vector.tensor_add(cp[:, i * 512:(i + 1) * 512],
                                             cp[:, i * 512:(i + 1) * 512],
                                             rblk[kc][:, qoff:qoff + 512])
                else:
                    for i in range(2):
                        kc = 2 * kb + i
                        nc.tensor.matmul(cp[:, i * 512:(i + 1) * 512],
                                         ident_p[:],
                                         rblk[kc][:, qoff:qoff + 512],
                                         start=False, stop=True)
                if prev is not None:
                    for i in range(2):
                        pv_mm(prev, po_t, 2 * kb + i)
                nc.scalar.activation(
                    pt_t[:, kb * 1024:(kb + 1) * 1024], cp[:],
                    AF.Exp, scale=1.0 / math.sqrt(HD))
                # prefetch: 2 reads/kb over kb 4-7 of qg 0/2 and kb 0-3 of
                # qg 1/3 -> the next 16-read half lands ~1.5 groups early.
                if nxt_rblk is not None and len(nxt_rblk) < 16:
                    if (qg % 2 == 0 and kb >= 4) or (qg % 2 == 1 and kb < 4):
                        base = len(nxt_rblk)
                        for kc2 in (base, base + 1):
                            nxt_rblk.append(issue_read(pf[0], pf[1], kc2))
                if hp2 == 0 and kb in (1, 4) and gi >= 1:
                    # spread next head-pair's rel-score units into hp0 groups
                    if r_done[1] < QT:
                        for hl2 in range(2):
                            r_unit(1, r_done[1], hl2, in_loop=True)
                        r_done[1] += 1
                if hp2 == 1 and hl == 1 and qg >= 2 and kb in (1, 3, 5, 7):
                    # output projection for already-normalized q ranges
                    d_unit(4 * (qg - 2) + (kb - 1) // 2)
            if prev is not None:
                normalize(prev, po_t)
            prev = (pt_t, lh, q0)
        # drain: last group's P@V and the remaining output tiles
        po_t = ps_o.tile([128, 512], F32, name="po", tag="po")
        for kc in range(16):
            pv_mm(prev, po_t, kc)
        normalize(prev, po_t)
        for qt in range(4 * (NB - 2), QT):
            d_unit(qt)
        cctx.close()

    nc.compile()
    return nc


_PROG_CACHE = {}


def _get_program(fast=True):
    if fast not in _PROG_CACHE:
        _PROG_CACHE[fast] = build_program(fast)
    return _PROG_CACHE[fast]


def _sinusoid_T():
    n = D // 2
    pos = np.arange(T - 1, -1, -1, dtype=np.float32)
    inv = np.exp(np.arange(n, dtype=np.float32)
                 * np.float32(-math.log(MAX_TS) / (n - 1)))
    st = pos[:, None] * inv[None, :]
    emb = np.concatenate([np.sin(st), np.cos(st)], axis=-1).astype(np.float32)
    return np.ascontiguousarray(emb.T)          # [D, T]


def make_in_maps(x, y, Wq, bq, Wk, bk, Wv, bv, Wp, bp, r_w_bias, r_r_bias,
                 Wo, bo, fast=True):
    in_dt = ml_dtypes.bfloat16 if fast else np.float32
    sinT = _sinusoid_T()
    rw = np.asarray(r_w_bias, np.float32).reshape(NH * HD)
    rr = np.asarray(r_r_bias, np.float32).reshape(NH * HD)
    in_maps = []
    for c in range(NCORES):
        b, g = divmod(c, HC)
        sl = slice(g * DC, (g + 1) * DC)
        m = {
            "xT": np.ascontiguousarray(x[b].T).astype(in_dt),
            "yT": np.ascontiguousarray(y[b].T).astype(in_dt),
            "sinT": sinT.astype(in_dt),
            "wq": np.ascontiguousarray(Wq[:, sl]).astype(in_dt),
            "wk": np.ascontiguousarray(Wk[:, sl]).astype(in_dt),
            "wv": np.ascontiguousarray(Wv[:, sl]).astype(in_dt),
            "wp": np.ascontiguousarray(Wp[:, sl]).astype(in_dt),
            "wo": np.ascontiguousarray(Wo[sl, :]),
            "bqw": np.ascontiguousarray((bq[sl] + rw[sl])[:, None]),
            "bqr": np.ascontiguousarray((bq[sl] + rr[sl])[:, None]),
            "bkb": np.ascontiguousarray(bk[sl][:, None]),
            "bpb": np.ascontiguousarray(bp[sl][:, None]),
            "bvb": np.ascontiguousarray(bv[sl][:, None]),
            "bob": (np.ascontiguousarray(np.tile(bo[None, :], (128, 1)))
                    if g == 0 else np.zeros((128, D), np.float32)),
        }
        in_maps.append({k: (v if v.dtype == in_dt else
                            np.asarray(v, np.float32))
                        for k, v in m.items()})
    return in_maps


def kernel(x, y, mask, Wq, bq, Wk, bk, Wv, bv, Wp, bp, r_w_bias, r_r_bias,
           Wo, bo, fast=True, trace=False, tmpdir=None):
    del mask  # all-ones by construction in this problem
    nc = _get_program(fast)
    in_maps = make_in_maps(
        np.asarray(x, np.float32), np.asarray(y, np.float32),
        np.asarray(Wq, np.float32), np.asarray(bq, np.float32),
        np.asarray(Wk, np.float32), np.asarray(bk, np.float32),
        np.asarray(Wv, np.float32), np.asarray(bv, np.float32),
        np.asarray(Wp, np.float32), np.asarray(bp, np.float32),
        np.asarray(r_w_bias, np.float32), np.asarray(r_r_bias, np.float32),
        np.asarray(Wo, np.float32), np.asarray(bo, np.float32), fast=fast)
    res = run_bass_kernel_spmd(nc, in_maps, core_ids=list(range(NCORES)),
                               trace=trace, tmpdir=tmpdir)
    outp = np.zeros((B, T, D), np.float32)
    for c in range(NCORES):
        b = c // HC
        outp[b] += res.results[c]["out"]
    if trace:
        return outp, res
    return outp


# revision 19
# speedup vs baseline: 1.2131x; 1.2131x over previous
"""Relative-position (Transformer-XL style) attention on 8 trn2 NeuronCores.

Sharding: data-parallel over batch (2) x tensor-parallel over heads (16/4=4
heads per core). Each core computes its 4 heads' attention for its batch and
a partial output projection (Wo rows for its heads); the host sums the 4
partials per batch (the "all-reduce").

Layout notes (per core, T=2048, D=1024, HC=4 local heads, HD=64):
  - q/k/pe are computed TRANSPOSED: [256, T] with head-dim on partitions,
    stored as 2 tiles [128, T] (head pair hp: heads 2hp at partitions 0-63,
    2hp+1 at 64-127).  Biases are per-partition there and folded into the
    PSUM->SBUF copies.  r_w/r_r biases are combined with bq on the host.
  - relative_shift: scores R = (q+r_r)@peT are written to a padded DRAM
    buffer P[T, T+1] (col 0 zeroed); the shifted matrix is exactly
    P_flat[(i+1)*T + j], i.e. a re-strided read (row stride T over a buffer
    of row pitch T+1).
  - content scores go straight to PSUM; the shifted rel scores are
    accumulated on top with an identity matmul (PE), then one fused
    exp(0.125*x) pass on ScalarE produces P (prob) tiles in bf16.
  - P@V: P tiles are [keys, queries] so v-as-lhsT matmuls produce
    outT [64, q] directly; a ones column appended to v also produces the
    softmax row-sums.  outT is normalized by 1/rowsum (partition-broadcast).
  - final: out_part[t, :] = sum_dc outT[dc, t] * Wo[dc, :] (+ bo on the
    g==0 cores), DMA'd out; host sums partials per batch.

All matmul operands are bf16 (PE streams at full rate, lower energy than
float32r); accumulation stays fp32 in PSUM.  fast=False is a full-fp32
reference build.
"""

import math
from contextlib import ExitStack

import ml_dtypes
import numpy as np

import concourse.bass as bass
import concourse.mybir as mybir
import concourse.tile as tile
from concourse.tile_rust import add_dep_helper
from concourse import bacc
from concourse.bass_utils import run_bass_kernel_spmd
from concourse.masks import make_identity

F32 = mybir.dt.float32
BF16 = mybir.dt.bfloat16
FP8 = mybir.dt.float8e3          # e3m4: 4 mantissa bits, range +-30
AF = mybir.ActivationFunctionType

B, T, D, NH = 2, 2048, 1024, 16
HD = D // NH              # 64 head dim
HC = 4                    # heads per core
DC = HC * HD              # 256 cols per core
NCORES = 8
KC = D // 128             # 8 contraction chunks for projections
NB = T // 512             # 4 key blocks of 512
QT = T // 128             # 16 q tiles
MAX_TS = 10000.0


def build_program(fast=True):
    P_DT = BF16 if fast else F32   # prob / v / pT dtype
    R_DT = BF16                    # rel-score DRAM roundtrip dtype

    nc = bacc.Bacc("TRN2", target_bir_lowering=False, debug=False,
                   num_devices=NCORES)

    IN_DT = BF16 if fast else F32  # streamed projection operand dtype
    # score-matmul operands (q/k/pe).  bf16: fp8 was measured to push the
    # result past the 2e-2 error gate (scores see ~4% noise amplified
    # through exp), and did not reduce the power throttle either.
    A_DT = BF16 if fast else F32   # engine-produced activation dtype
    xT = nc.dram_tensor("xT", [D, T], IN_DT, kind="ExternalInput")
    yT = nc.dram_tensor("yT", [D, T], IN_DT, kind="ExternalInput")
    sinT = nc.dram_tensor("sinT", [D, T], IN_DT, kind="ExternalInput")
    wq = nc.dram_tensor("wq", [D, DC], IN_DT, kind="ExternalInput")
    wk = nc.dram_tensor("wk", [D, DC], IN_DT, kind="ExternalInput")
    wv = nc.dram_tensor("wv", [D, DC], IN_DT, kind="ExternalInput")
    wp = nc.dram_tensor("wp", [D, DC], IN_DT, kind="ExternalInput")
    wo = nc.dram_tensor("wo", [DC, D], F32, kind="ExternalInput")
    bqw = nc.dram_tensor("bqw", [DC, 1], F32, kind="ExternalInput")
    bqr = nc.dram_tensor("bqr", [DC, 1], F32, kind="ExternalInput")
    bkb = nc.dram_tensor("bkb", [DC, 1], F32, kind="ExternalInput")
    bpb = nc.dram_tensor("bpb", [DC, 1], F32, kind="ExternalInput")
    bvb = nc.dram_tensor("bvb", [DC, 1], F32, kind="ExternalInput")
    bob = nc.dram_tensor("bob", [128, D], F32, kind="ExternalInput")
    out = nc.dram_tensor("out", [T, D], F32, kind="ExternalOutput")

    with tile.TileContext(nc) as tc, ExitStack() as ctx:
        persist = ctx.enter_context(tc.tile_pool(name="persist", bufs=1))
        dpool = ctx.enter_context(tc.tile_pool(name="dram", bufs=1,
                                               space="DRAM"))

        # ---- constants / biases -------------------------------------------
        ident_p = persist.tile([128, 128], P_DT, tag="ident_p")
        make_identity(nc, ident_p[:])
        ident_f = persist.tile([128, 128], F32, tag="ident_f")
        make_identity(nc, ident_f[:])

        def load_bias_pair(name, dram):
            ts_ = [persist.tile([128, 1], F32, name=f"{name}{i}",
                                tag=f"{name}{i}") for i in range(2)]
            for i in range(2):
                nc.scalar.dma_start(ts_[i][:], dram[i * 128:(i + 1) * 128, 0:1])
            return ts_

        bqw_sb = load_bias_pair("bqw", bqw)
        bqr_sb = load_bias_pair("bqr", bqr)
        bk_sb = load_bias_pair("bk", bkb)
        bp_sb = load_bias_pair("bp", bpb)
        bv_sb = load_bias_pair("bv", bvb)
        bo_sb = persist.tile([128, D], F32, tag="bo")
        nc.scalar.dma_start(bo_sb[:], bob[:, :])

        wo_sb = [persist.tile([HD, D], P_DT if fast else F32,
                              name=f"wo{i}", tag=f"wo{i}")
                 for i in range(HC)]
        with tc.tile_pool(name="wotmp", bufs=2) as wotmp:
            for i in range(HC):
                wo_f = wotmp.tile([HD, D], F32, name=f"wof{i}", tag="wof")
                nc.scalar.dma_start(wo_f[:], wo[i * HD:(i + 1) * HD, :])
                nc.scalar.copy(wo_sb[i][:], wo_f[:])

        # ---- DRAM scratch for the relative-shift roundtrip ----------------
        pbufs = [dpool.tile([T, T + 1], R_DT, name=f"pbuf{h}",
                            tag=f"pbuf{h}") for h in range(HC)]
        pbuf_h = [pb[:].tensor for pb in pbufs]

        w_insts = {}             # (h, qt) -> rel-score write DMA

        # ---- persistent activations ---------------------------------------
        cqT = [persist.tile([128, T], A_DT, name=f"cqT{i}", tag=f"cqT{i}")
               for i in range(2)]
        rqT = [persist.tile([128, T], A_DT, name=f"rqT{i}", tag=f"rqT{i}")
               for i in range(2)]
        kT = [persist.tile([128, T], A_DT, name=f"kT{i}", tag=f"kT{i}")
              for i in range(2)]
        peT = [persist.tile([128, T], A_DT, name=f"peT{i}", tag=f"peT{i}")
               for i in range(2)]
        # v with a ones column appended per head: [keys, (kc, head, 65)];
        # the ones row makes P@V also produce the softmax row-sums.
        VW = HC * (HD + 1)
        v_sb = persist.tile([128, QT * VW], P_DT, tag="v")
        O_DT = P_DT if fast else F32
        outT = [persist.tile([HD, T], O_DT, name=f"outT{i}", tag=f"outT{i}")
                for i in range(HC)]

        # ---- rel-score (R) infrastructure ---------------------------------
        # created before projections so R units can interleave into them
        rsb_pool = ctx.enter_context(tc.tile_pool(name="rsb", bufs=2))
        ps_r = ctx.enter_context(tc.tile_pool(name="psr", bufs=1, space="PSUM"))
        o_pool = ctx.enter_context(tc.tile_pool(name="osb", bufs=2))

        def r_unit(hp2, qt, hl, in_loop=False):
            """rel scores for one (head, q-tile) -> padded DRAM buf.

            The buffer holds exp(rel/sqrt(HD)) rather than raw rel scores:
            exp(c+r) = exp(c)*exp(r), so the shifted values multiply into
            the exp'd content scores on DVE instead of accumulating via an
            identity matmul on the (throttled) PE.  The PSUM->SBUF copy
            becomes the exp, so this costs nothing extra on ScalarE here.
            """
            h = 2 * hp2 + hl
            pb, pe_ = 64 * hl, 64 * (hl + 1)
            # [128, T+1]: col 0 is the pad column of the shift buffer --
            # exp(0) = 1 -- so the write covers whole pitch-(T+1) rows.
            rsb_t = rsb_pool.tile([128, T + 1], R_DT, name="rsb_t", tag="rsb")
            nc.vector.memset(rsb_t[:, 0:1], 1.0)
            for jb in range(2):
                rp = ps_r.tile([128, 1024], F32, name="rp", tag="rp")
                for i in range(2):
                    nc.tensor.matmul(
                        rp[:, i * 512:(i + 1) * 512],
                        rqT[hp2][pb:pe_, qt * 128:(qt + 1) * 128],
                        peT[hp2][pb:pe_,
                                 (2 * jb + i) * 512:(2 * jb + i + 1) * 512],
                        start=True, stop=True)
                nc.scalar.activation(
                    rsb_t[:, 1 + jb * 1024:1 + (jb + 1) * 1024], rp[:],
                    AF.Exp, scale=1.0 / math.sqrt(HD))
            dst = bass.AP(tensor=pbuf_h[h], offset=qt * 128 * (T + 1),
                          ap=[[T + 1, 128], [1, T + 1]])
            w_insts[(h, qt)] = nc.gpsimd.dma_start(dst, rsb_t[:])

        r_done = [0, 0]          # next unemitted qt per head pair

        def ensure_r(hp2, upto_qt, in_loop=False):
            while r_done[hp2] <= min(upto_qt, QT - 1):
                for hl in range(2):
                    r_unit(hp2, r_done[hp2], hl, in_loop=in_loop)
                r_done[hp2] += 1

        def r_stepper(hp2):
            def step():
                if r_done[hp2] < QT:
                    for hl in range(2):
                        r_unit(hp2, r_done[hp2], hl)
                    r_done[hp2] += 1
            return step

        # ---- phase B: projections -----------------------------------------
        def proj_T(src_dram, w_dram, dests, interleave=None):
            """dest[mc][:, :] = (W[:, mc]^T @ src) + bias  (transposed proj).

            Stream kept resident; two nb-passes over 2x[128,1024] PSUM tiles
            (6 banks with bufs=3, leaving room for concurrent R units).
            """
            with ExitStack() as c2:
                wpool = c2.enter_context(tc.tile_pool(name="wproj", bufs=1))
                spool = c2.enter_context(tc.tile_pool(name="sproj", bufs=1))
                pspool = c2.enter_context(
                    tc.tile_pool(name="psproj", bufs=3, space="PSUM"))
                w_t = [wpool.tile([128, DC], IN_DT, name=f"w{k}", tag=f"w{k}")
                       for k in range(KC)]
                s_t = [spool.tile([128, T], IN_DT, name=f"s{k}", tag=f"s{k}")
                       for k in range(KC)]
                for k in range(KC):
                    nc.scalar.dma_start(w_t[k][:],
                                        w_dram[k * 128:(k + 1) * 128, :])
                    eng = nc.sync if k % 2 == 0 else nc.scalar
                    eng.dma_start(s_t[k][:], src_dram[k * 128:(k + 1) * 128, :])
                for p2 in range(2):
                    ps = [pspool.tile([128, 1024], F32, name=f"ps{mc}",
                                      tag="ps") for mc in range(2)]
                    for k in range(KC):
                        for mc in range(2):
                            for i in range(2):
                                nb = 2 * p2 + i
                                nc.tensor.matmul(
                                    ps[mc][:, i * 512:(i + 1) * 512],
                                    w_t[k][:, mc * 128:(mc + 1) * 128],
                                    s_t[k][:, nb * 512:(nb + 1) * 512],
                                    start=(k == 0), stop=(k == KC - 1))
                        if interleave is not None:
                            interleave()
                    for mc in range(2):
                        for d_tile, d_bias in dests:
                            nc.scalar.activation(
                                d_tile[mc][:, 2 * p2 * 512:(2 * p2 + 2) * 512],
                                ps[mc][:], AF.Identity,
                                bias=d_bias[mc][:], scale=1.0)

        proj_T(xT, wq, [(cqT, bqw_sb), (rqT, bqr_sb)])
        proj_T(sinT, wp, [(peT, bp_sb)])
        proj_T(yT, wk, [(kT, bk_sb)], interleave=r_stepper(0))

        # vT = (Wv^T @ yT) + bv, then PE-transpose into natural [t, dc] layout
        with ExitStack() as c2:
            vtpool = c2.enter_context(tc.tile_pool(name="vtp", bufs=1))
            vT = [vtpool.tile([128, T], F32, name=f"vT{i}", tag=f"vT{i}")
                  for i in range(2)]
            proj_T(yT, wv, [(vT, bv_sb)], interleave=r_stepper(0))
            pst_v = c2.enter_context(
                tc.tile_pool(name="pstv", bufs=4, space="PSUM"))
            v_view = v_sb[:].rearrange("p (tt h d) -> p tt h d",
                                       h=HC, d=HD + 1)
            nc.vector.memset(v_view[:, :, :, HD:HD + 1], 1.0)
            for mc in range(2):
                for tt in range(QT):
                    tpv = pst_v.tile([128, 128], F32)
                    nc.tensor.matmul(tpv[:], vT[mc][:, tt * 128:(tt + 1) * 128],
                                     ident_f[:], is_transpose=True)
                    dst = v_view[:, tt, 2 * mc:2 * mc + 2, 0:HD]
                    srcv = tpv[:].rearrange("p (a b) -> p a b", b=HD)
                    if tt % 2 == 0:
                        nc.scalar.copy(dst, srcv)
                    else:
                        nc.vector.tensor_copy(dst, srcv)

        # ---- phase C: attention, software-pipelined -----------------------
        cctx = ExitStack()
        rst_pool = cctx.enter_context(tc.tile_pool(name="rst", bufs=40))
        pt_pool = cctx.enter_context(tc.tile_pool(name="pt", bufs=2))
        small_pool = cctx.enter_context(tc.tile_pool(name="small", bufs=2))
        ps_c = cctx.enter_context(tc.tile_pool(name="psc", bufs=2, space="PSUM"))
        ps_o = cctx.enter_context(tc.tile_pool(name="pso", bufs=2, space="PSUM"))

        def pv_mm(prev, po_t, kc):
            pt_p, plh, _ = prev
            nc.tensor.matmul(
                po_t[0:HD + 1, :],
                v_sb[:, kc * VW + plh * (HD + 1):kc * VW + (plh + 1) * (HD + 1)],
                pt_p[:, kc * 512:(kc + 1) * 512],
                start=(kc == 0), stop=(kc == 15))

        def normalize(prev, po_t):
            # NOTE: ACT Ln+Exp for 1/Z thrashes the ACT function-table (the
            # set loader alternates sets, ~1.3us per swap) — keep reciprocal
            # on DVE even though the [1,512] single-lane shape is slow.
            _, plh, pq0 = prev
            rec = small_pool.tile([1, 512], F32, name="rec", tag="rec")
            nc.vector.reciprocal(rec[:], po_t[HD:HD + 1, :])
            rec_b = small_pool.tile([HD, 512], F32, name="rec_b", tag="rec_b")
            nc.gpsimd.partition_broadcast(rec_b[:], rec[:])
            nc.vector.tensor_mul(outT[plh][:, pq0:pq0 + 512],
                                 po_t[0:HD, :], rec_b[:])

        def d_unit(qt):
            """partial output projection for one q-tile (uses a psc slot)."""
            wps = ps_c.tile([128, 1024], F32, name="wps", tag="cp")
            for nb2 in range(2):
                for lh2 in range(HC):
                    nc.tensor.matmul(
                        wps[:, nb2 * 512:(nb2 + 1) * 512],
                        outT[lh2][:, qt * 128:(qt + 1) * 128],
                        wo_sb[lh2][:, nb2 * 512:(nb2 + 1) * 512],
                        start=(lh2 == 0), stop=(lh2 == HC - 1))
            o_t = o_pool.tile([128, D], F32, name="o_t", tag="o_t")
            for nb2 in range(2):
                nc.vector.tensor_add(o_t[:, nb2 * 512:(nb2 + 1) * 512],
                                     wps[:, nb2 * 512:(nb2 + 1) * 512],
                                     bo_sb[:, nb2 * 512:(nb2 + 1) * 512])
            nc.gpsimd.dma_start(out[qt * 128:(qt + 1) * 128, :], o_t[:])

        # The P@V matmuls of the previous group interleave into the score
        # loop of the current one.  hl-outer order so consecutive qg groups
        # share one head's DRAM buf and each transposed read covers 1024 q
        # rows (2 groups).  The next head pair's R units and the output
        # projection of finished q ranges are spread into the loop.
        def issue_read(h2, half, kc):
            q0p = half * 1024
            rst = rst_pool.tile([128, 1024], R_DT, name=f"rst{kc}", tag="rst")
            src = bass.AP(tensor=pbuf_h[h2],
                          offset=(q0p + 1) * T + kc * 128,
                          ap=[[T, 1024], [1, 128]])
            rd = nc.sync.dma_start_transpose(rst[:], src)
            for qt2 in range(8 * half, min(8 * half + 9, QT)):
                add_dep_helper(rd.ins, w_insts[(h2, qt2)].ins,
                               reason="shifted read after rel write")
            return rst

        ensure_r(0, 8)
        groups = [(hp2, hl, qg)
                  for hp2 in range(2) for hl in range(2) for qg in range(NB)]
        prev = None
        rblk = [None] * 16
        nxt_rblk = None
        for gi, (hp2, hl, qg) in enumerate(groups):
            h = 2 * hp2 + hl
            lh = 2 * hp2 + hl            # local head index 0..3
            q0 = qg * 512
            pb, pe_ = 64 * hl, 64 * (hl + 1)
            if qg % 2 == 0:
                if nxt_rblk is not None:
                    rblk = nxt_rblk
                    nxt_rblk = None
                else:
                    ensure_r(hp2, 8 * (qg // 2) + 8)
                    rblk = [issue_read(h, qg // 2, kc) for kc in range(16)]
            # what the 2-group window after this one will need: at qg 0/1
            # prefetch (h, half 1); at qg 2/3 prefetch the next head's half 0.
            pf = None
            if qg in (0, 1):
                pf = (h, 1)
            elif gi + 4 - qg < len(groups):
                nhp, nhl, _ = groups[gi + 4 - qg]
                pf = (2 * nhp + nhl, 0)
            if pf is not None and nxt_rblk is None and qg in (0, 2):
                ensure_r(pf[0] // 2, 8 * pf[1] + 8, in_loop=True)
                nxt_rblk = []
            qoff = (qg % 2) * 512
            pt_t = pt_pool.tile([128, 16 * 512], P_DT, name="pt_t", tag="pt")
            po_t = None
            if prev is not None:
                po_t = ps_o.tile([128, 512], F32, name="po", tag="po")
            for kb in range(8):
                cp = ps_c.tile([128, 1024], F32, name="cp", tag="cp")
                # content scores only on PE; the shifted exp'd rel factors
                # multiply in on DVE after the exp, so the PE stream never
                # depends on the DMA-transposed rel reads.
                for i in range(2):
                    kc = 2 * kb + i
                    nc.tensor.matmul(
                        cp[:, i * 512:(i + 1) * 512],
                        kT[hp2][pb:pe_, kc * 128:(kc + 1) * 128],
                        cqT[hp2][pb:pe_, q0:q0 + 512],
                        start=True, stop=True)
                if prev is not None:
                    for i in range(2):
                        pv_mm(prev, po_t, 2 * kb + i)
                nc.scalar.activation(
                    pt_t[:, kb * 1024:(kb + 1) * 1024], cp[:],
                    AF.Exp, scale=1.0 / math.sqrt(HD))
                for i in range(2):
                    kc = 2 * kb + i
                    nc.vector.tensor_mul(
                        pt_t[:, kc * 512:(kc + 1) * 512],
                        pt_t[:, kc * 512:(kc + 1) * 512],
                        rblk[kc][:, qoff:qoff + 512])
                # prefetch: 2 reads/kb over kb 4-7 of qg 0/2 and kb 0-3 of
                # qg 1/3 -> the next 16-read half lands ~1.5 groups early.
                if nxt_rblk is not None and len(nxt_rblk) < 16:
                    if (qg % 2 == 0 and kb >= 4) or (qg % 2 == 1 and kb < 4):
                        base = len(nxt_rblk)
                        for kc2 in (base, base + 1):
                            nxt_rblk.append(issue_read(pf[0], pf[1], kc2))
                if hp2 == 0 and kb in (1, 4) and gi >= 1:
                    # spread next head-pair's rel-score units into hp0 groups
                    if r_done[1] < QT:
                        for hl2 in range(2):
                            r_unit(1, r_done[1], hl2, in_loop=True)
                        r_done[1] += 1
                if hp2 == 1 and hl == 1 and qg >= 2 and kb in (1, 3, 5, 7):
                    # output projection for already-normalized q ranges
                    d_unit(4 * (qg - 2) + (kb - 1) // 2)
            if prev is not None:
                normalize(prev, po_t)
            prev = (pt_t, lh, q0)
        # drain: last group's P@V and the remaining output tiles
        po_t = ps_o.tile([128, 512], F32, name="po", tag="po")
        for kc in range(16):
            pv_mm(prev, po_t, kc)
        normalize(prev, po_t)
        for qt in range(4 * (NB - 2), QT):
            d_unit(qt)
        cctx.close()

    nc.compile()
    return nc


_PROG_CACHE = {}


def _get_program(fast=True):
    if fast not in _PROG_CACHE:
        _PROG_CACHE[fast] = build_program(fast)
    return _PROG_CACHE[fast]


def _sinusoid_T():
    n = D // 2
    pos = np.arange(T - 1, -1, -1, dtype=np.float32)
    inv = np.exp(np.arange(n, dtype=np.float32)
                 * np.float32(-math.log(MAX_TS) / (n - 1)))
    st = pos[:, None] * inv[None, :]
    emb = np.concatenate([np.sin(st), np.cos(st)], axis=-1).astype(np.float32)
    return np.ascontiguousarray(emb.T)          # [D, T]


def make_in_maps(x, y, Wq, bq, Wk, bk, Wv, bv, Wp, bp, r_w_bias, r_r_bias,
                 Wo, bo, fast=True):
    in_dt = ml_dtypes.bfloat16 if fast else np.float32
    sinT = _sinusoid_T()
    rw = np.asarray(r_w_bias, np.float32).reshape(NH * HD)
    rr = np.asarray(r_r_bias, np.float32).reshape(NH * HD)
    in_maps = []
    for c in range(NCORES):
        b, g = divmod(c, HC)
        sl = slice(g * DC, (g + 1) * DC)
        m = {
            "xT": np.ascontiguousarray(x[b].T).astype(in_dt),
            "yT": np.ascontiguousarray(y[b].T).astype(in_dt),
            "sinT": sinT.astype(in_dt),
            "wq": np.ascontiguousarray(Wq[:, sl]).astype(in_dt),
            "wk": np.ascontiguousarray(Wk[:, sl]).astype(in_dt),
            "wv": np.ascontiguousarray(Wv[:, sl]).astype(in_dt),
            "wp": np.ascontiguousarray(Wp[:, sl]).astype(in_dt),
            "wo": np.ascontiguousarray(Wo[sl, :]),
            "bqw": np.ascontiguousarray((bq[sl] + rw[sl])[:, None]),
            "bqr": np.ascontiguousarray((bq[sl] + rr[sl])[:, None]),
            "bkb": np.ascontiguousarray(bk[sl][:, None]),
            "bpb": np.ascontiguousarray(bp[sl][:, None]),
            "bvb": np.ascontiguousarray(bv[sl][:, None]),
            "bob": (np.ascontiguousarray(np.tile(bo[None, :], (128, 1)))
                    if g == 0 else np.zeros((128, D), np.float32)),
        }
        in_maps.append({k: (v if v.dtype == in_dt else
                            np.asarray(v, np.float32))
                        for k, v in m.items()})
    return in_maps


def kernel(x, y, mask, Wq, bq, Wk, bk, Wv, bv, Wp, bp, r_w_bias, r_r_bias,
           Wo, bo, fast=True, trace=False, tmpdir=None):
    del mask  # all-ones by construction in this problem
    nc = _get_program(fast)
    in_maps = make_in_maps(
        np.asarray(x, np.float32), np.asarray(y, np.float32),
        np.asarray(Wq, np.float32), np.asarray(bq, np.float32),
        np.asarray(Wk, np.float32), np.asarray(bk, np.float32),
        np.asarray(Wv, np.float32), np.asarray(bv, np.float32),
        np.asarray(Wp, np.float32), np.asarray(bp, np.float32),
        np.asarray(r_w_bias, np.float32), np.asarray(r_r_bias, np.float32),
        np.asarray(Wo, np.float32), np.asarray(bo, np.float32), fast=fast)
    res = run_bass_kernel_spmd(nc, in_maps, core_ids=list(range(NCORES)),
                               trace=trace, tmpdir=tmpdir)
    outp = np.zeros((B, T, D), np.float32)
    for c in range(NCORES):
        b = c // HC
        outp[b] += res.results[c]["out"]
    if trace:
        return outp, res
    return outp
